# revision 1
# baseline (speedup 1.0000x reference)
"""Sharded cosine-similarity kNN (k=5) + weighted centroid on 8 TRN2 NeuronCores.

Strategy (standard sharded kNN):
  - Shard the 500000x768 database row-wise across 8 cores (62500 rows each,
    padded to 62592 rows = 489 column-groups of [128 rows x 768], padding with
    copies of -query, whose cosine similarity is ~-1 and can never enter the
    top-k). Shards are quantized to bf16 on the host: halves the HBM stream
    (~96 MB/core); exact f32 precision is restored host-side by re-scoring
    the top-64 candidates against the original f32 rows (device sim error
    ~5e-4 vs a ~1e-2 gap between ranks 5 and 20 — selection is safe).
  - Each core streams its shard from HBM once. DMA-only sweeps show the
    stream rate is partition-line-size sensitive (needs ~24 KB lines for
    ~335 GB/s/core), so chunk size G is chosen to keep [128, G*768] transfers
    at 24 KB/partition, with small final chunks (less non-overlapped compute
    after the last DMA) and the 1-column padded-tail tile emitted first.
  - Per 768-column row-group: DVE scalar_tensor_tensor (bypass,mult,
    accum_out) computes dot(row, q); ACT activation(Square, accum_out)
    computes ||row||^2. Every 16th square runs on DVE instead to balance
    engine busy (~7.6 us/tile vs 9.0 us DMA per G=8 tile).
  - Epilogue (sims = dots / max(sqrt(n2), eps)) is emitted in 64-column
    chunks interleaved with the stream so only ~17 columns + top-8
    max/max_index + an 8-KB output DMA remain after the last tile.
  - Host: gather 8x128x8 candidates, divide by ||q|| (order-preserving),
    global top-5, inverse-square-distance weights, tiny centroid gather.

Environment workaround: this container's walrus build rejects any instruction
with more than one semaphore wait; see split_sync_waits() below.
"""

import contextlib

import ml_dtypes
import numpy as np

import concourse.bass as bass
import concourse.mybir as mybir
from concourse.tile import TileContext
from concourse.bass_utils import run_bass_kernel_spmd

N_CORES = 8
D = 768
N_ROWS = 500000
SHARD = N_ROWS // N_CORES   # 62500
P = 128
NCOLS = 489                 # 62592 padded rows / 128
PAD_ROWS = NCOLS * P        # 62592 (92 pad rows)
GMAX = 8
K = 5
COS_EPS = 1e-8
W_EPS = 1e-6

# (col0, g) chunks in DMA-emission order: the padded-tail column first (keeps
# it off the critical end), then 60 G=8 chunks, then two G=4 chunks so the
# last tile's compute shadow is short. Column c covers shard rows
# col0*128 + p*g + (c - col0) for partition p.
CHUNK_PLAN = (
    [(488, 1)]
    + [(i * 8, 8) for i in range(60)]
    + [(480, 4), (484, 2), (486, 2)]
)
assert sorted(c for c0, g in CHUNK_PLAN for c in range(c0, c0 + g)) == list(
    range(NCOLS)
)

_f32 = mybir.dt.float32
_bf16 = mybir.dt.bfloat16
_u32 = mybir.dt.uint32

_wsplit_ctr = [0]


def split_sync_waits(nc):
    """Workaround for this container's walrus build: it rejects any instruction
    carrying more than ONE semaphore wait ("Too many sync wait commands" in
    setupSyncWait during codegen). Tile's scheduler freely attaches several
    waits to one instruction, so after TileContext scheduling we split them:
    every instruction keeps its last wait, and each extra wait is hoisted onto
    its own NoOp placed immediately before it in the same basic block (same
    engine, so program order preserves wait-before-execute semantics)."""
    for f in nc.m.functions:
        for b in f.blocks:
            needs_fix = any(
                getattr(i, "sync_info", None) is not None
                and i.sync_info.on_wait
                and len(i.sync_info.on_wait) > 1
                for i in b.instructions
            )
            if not needs_fix:
                continue
            new_insts = []
            for inst in b.instructions:
                si = getattr(inst, "sync_info", None)
                if si is not None and si.on_wait and len(si.on_wait) > 1:
                    waits = list(si.on_wait)
                    for w in waits[:-1]:
                        _wsplit_ctr[0] += 1
                        nop = mybir.InstNoOp(
                            name=f"WSPLIT-{_wsplit_ctr[0]}", ins=[], outs=[]
                        )
                        nop.engine = inst.engine
                        nop.sync_info = mybir.SyncInfo(on_wait=[w], on_update=[])
                        new_insts.append(nop)
                    inst.sync_info = mybir.SyncInfo(
                        on_wait=[waits[-1]], on_update=list(si.on_update or [])
                    )
                new_insts.append(inst)
            b.instructions[:] = new_insts
    return nc


def build_nc(db_bufs: int = 6, repeat: int = 1, dve_sq_every: int = 16,
             epi_every: int = 64, out_space: str = "SBUF",
             rings: tuple = ("sync",), chunk_plan: list | None = None,
             aux_ring: str = "scalar", gp_sq_every: int = 0,
             dots_mode: str = "stt"):
    """repeat>1 wraps the body in tc.For_i for on-device timing (one NEFF).
    dve_sq_every=k moves every k-th square op from ACT to DVE (0 = all ACT).
    epi_every: emit the sims epilogue for each completed chunk of this many
    columns, interleaved with the DMA stream. out_space: where the unused
    elementwise outputs of the accumulating ops live ("PSUM" keeps their
    write streams off the SBUF ports the DMA stream needs). rings: HWDGE
    issue engines cycled per chunk ("sync"/"scalar")."""
    if chunk_plan is None:
        chunk_plan = CHUNK_PLAN
    gmax = max(g for _, g in chunk_plan)
    nc = bass.Bass()
    qrep_w = D if dots_mode == "stt" else gmax * D
    db = nc.dram_tensor("db", [PAD_ROWS * D], _bf16, kind="ExternalInput")
    qrep = nc.dram_tensor("qrep", [P, qrep_w], _bf16, kind="ExternalInput")
    outv = nc.dram_tensor("outv", [P, 8], _f32, kind="ExternalOutput")
    outi = nc.dram_tensor("outi", [P, 8], _u32, kind="ExternalOutput")

    ew_bufs = 3 if out_space == "SBUF" else 2
    with TileContext(nc) as tc:
        with (
            tc.tile_pool(name="persist", bufs=1) as persist,
            tc.tile_pool(name="dbp", bufs=db_bufs) as dbp,
            tc.tile_pool(name="dv", bufs=ew_bufs, space=out_space) as dvp,
            tc.tile_pool(name="da", bufs=ew_bufs, space=out_space) as dap,
        ):
            # qt and the tiny result DMAs ride the aux (scalar-engine HWDGE)
            # ring: their waits depend on compute, and on the sync ring they
            # would stall the in-order db stream (visible as a ~10us/iter
            # bubble in the repeat-loop proxy).
            aux = getattr(nc, aux_ring)
            loop = tc.For_i(0, repeat, 1) if repeat > 1 else contextlib.nullcontext()
            with loop:
                qt = persist.tile([P, qrep_w], _bf16, tag="qt")
                aux.dma_start(qt[:], qrep[:])

                dots = persist.tile([P, NCOLS], _f32, tag="dots")
                n2 = persist.tile([P, NCOLS], _f32, tag="n2")
                dn = persist.tile([P, NCOLS], _f32, tag="dn")
                inv = persist.tile([P, NCOLS], _f32, tag="inv")
                sims = persist.tile([P, NCOLS], _f32, tag="sims")

                nproc = [0]

                def process(sb_ap, col):
                    if dots_mode == "stt":
                        tout = dvp.tile([P, D], _bf16, tag="tout")
                        nc.vector.scalar_tensor_tensor(
                            out=tout[:],
                            in0=sb_ap,
                            scalar=0.0,
                            in1=qt[:, :D],
                            op0=mybir.AluOpType.bypass,
                            op1=mybir.AluOpType.mult,
                            accum_out=dots[:, col : col + 1],
                        )
                    nproc[0] += 1
                    # ACT square+accum (~1.0us) vs DVE dot (~0.9us): shifting
                    # every 16th square to DVE balances both engines; GPSIMD
                    # can take a further share as a third (slow) engine.
                    if gp_sq_every and nproc[0] % gp_sq_every == 0:
                        gq = dap.tile([P, D], _bf16, tag="gout")
                        nc.gpsimd.scalar_tensor_tensor(
                            out=gq[:],
                            in0=sb_ap,
                            scalar=0.0,
                            in1=sb_ap,
                            op0=mybir.AluOpType.bypass,
                            op1=mybir.AluOpType.mult,
                            accum_out=n2[:, col : col + 1],
                        )
                    elif dve_sq_every and nproc[0] % dve_sq_every == 0:
                        sq = dvp.tile([P, D], _bf16, tag="tout")
                        nc.vector.scalar_tensor_tensor(
                            out=sq[:],
                            in0=sb_ap,
                            scalar=0.0,
                            in1=sb_ap,
                            op0=mybir.AluOpType.bypass,
                            op1=mybir.AluOpType.mult,
                            accum_out=n2[:, col : col + 1],
                        )
                    else:
                        aout = dap.tile([P, D], _bf16, tag="aout")
                        nc.scalar.activation(
                            out=aout[:],
                            in_=sb_ap,
                            func=mybir.ActivationFunctionType.Square,
                            accum_out=n2[:, col : col + 1],
                        )

                def epilogue_chunk(lo, hi):
                    if hi <= lo:
                        return
                    nc.scalar.sqrt(dn[:, lo:hi], n2[:, lo:hi])
                    nc.vector.tensor_scalar_max(dn[:, lo:hi], dn[:, lo:hi], COS_EPS)
                    nc.vector.reciprocal(inv[:, lo:hi], dn[:, lo:hi])
                    nc.vector.tensor_mul(sims[:, lo:hi], dots[:, lo:hi],
                                         inv[:, lo:hi])

                epi_done = [0]

                def maybe_epilogue(complete_cols):
                    # columns [0, complete_cols) are fully accumulated
                    hi = (complete_cols // epi_every) * epi_every
                    if hi > epi_done[0]:
                        epilogue_chunk(epi_done[0], hi)
                        epi_done[0] = hi

                for ti, (col0, g) in enumerate(chunk_plan):
                    free = g * D
                    sb = dbp.tile([P, gmax * D], _bf16, tag="sb")
                    src = db[col0 * P * D : (col0 + g) * P * D]
                    eng = getattr(nc, rings[ti % len(rings)])
                    eng.dma_start(
                        sb[:, :free], src.rearrange("(p f) -> p f", f=free)
                    )
                    if dots_mode == "tt":
                        # one fused multiply + one segmented reduce per chunk
                        # instead of g per-column STT ops
                        prod = dvp.tile([P, gmax * D], _bf16, tag="prod")
                        nc.vector.tensor_mul(
                            prod[:, :free], sb[:, :free], qt[:, :free]
                        )
                        nc.vector.reduce_sum(
                            out=dots[:, col0 : col0 + g],
                            in_=prod[:, :free].rearrange("p (g d) -> p g d", d=D),
                            axis=mybir.AxisListType.X,
                        )
                    for j in range(g):
                        process(sb[:, j * D : (j + 1) * D], col0 + j)
                    if col0 != 488:
                        maybe_epilogue(col0 + g)

                epilogue_chunk(epi_done[0], NCOLS)

                vals8 = persist.tile([P, 8], _f32, tag="vals8")
                idx8 = persist.tile([P, 8], _u32, tag="idx8")
                nc.vector.max(vals8[:], sims[:])
                aux.dma_start(outv[:], vals8[:])
                nc.vector.max_index(idx8[:], vals8[:], sims[:])
                aux.dma_start(outi[:], idx8[:])
    split_sync_waits(nc)
    return nc


def _prep_inputs(query: np.ndarray, database: np.ndarray, n_cores: int = N_CORES,
                 shard: int = SHARD, q_tile: int = 1):
    """Build per-core input maps, quantized to bf16 (halves the HBM stream;
    exactness is restored host-side by re-scoring the top candidates against
    the original f32 rows). Pads each shard with copies of -query (cosine
    similarity ~-1: never selected)."""
    q = np.ascontiguousarray(np.asarray(query, dtype=np.float32)).reshape(1, D)
    db = np.asarray(database, dtype=np.float32)
    qrep = np.ascontiguousarray(
        np.tile(q, (P, q_tile)).astype(ml_dtypes.bfloat16)
    )  # [128, q_tile*768]
    in_maps = []
    for c in range(n_cores):
        sh = np.empty((PAD_ROWS, D), dtype=np.float32)
        sh[:shard] = db[c * shard : (c + 1) * shard]
        sh[shard:] = -q
        in_maps.append(
            {"db": sh.reshape(-1).astype(ml_dtypes.bfloat16), "qrep": qrep}
        )
    return in_maps


# per-column chunk lookup derived from CHUNK_PLAN
_COL0 = np.zeros(NCOLS, dtype=np.int64)
_CG = np.zeros(NCOLS, dtype=np.int64)
for _c0, _g in CHUNK_PLAN:
    _COL0[_c0 : _c0 + _g] = _c0
    _CG[_c0 : _c0 + _g] = _g


def _cols_to_rows(cols: np.ndarray, p_idx: np.ndarray) -> np.ndarray:
    """Map candidate column index (per partition) back to shard row:
    column c in chunk (col0, g) => row col0*128 + p*g + (c-col0)."""
    c0 = _COL0[cols]
    g = _CG[cols]
    return c0 * P + p_idx * g + (cols - c0)


def _host_reduce(results, query: np.ndarray, database: np.ndarray,
                 n_cores: int = N_CORES, shard: int = SHARD) -> np.ndarray:
    q = np.asarray(query, dtype=np.float32).reshape(1, D)
    db = np.asarray(database, dtype=np.float32)

    vals = np.stack([r["outv"] for r in results])          # [C,128,8] dot/||row||
    cols = np.stack([r["outi"] for r in results]).astype(np.int64)  # [C,128,8]

    c_idx = np.arange(n_cores, dtype=np.int64)[:, None, None]
    p_idx = np.arange(P, dtype=np.int64)[None, :, None]
    shard_row = _cols_to_rows(cols, p_idx)
    gidx = c_idx * shard + shard_row

    valid = (shard_row < shard).ravel()
    v = vals.ravel()[valid]
    g = gidx.ravel()[valid]

    # Device sims are bf16-precision (~5e-4): take a generous candidate pool
    # by device score, then re-score those rows exactly in f32/f64 so the
    # final top-5 and weights match the f32 reference bit-for-bit in practice.
    npool = min(64, v.size)
    pool = np.argpartition(-v, npool - 1)[:npool]
    cand = g[pool]

    qn = max(float(np.linalg.norm(q.astype(np.float64))), COS_EPS)
    rows = db[cand].astype(np.float64)
    dn_c = np.maximum(np.linalg.norm(rows, axis=1), COS_EPS)
    sims_c = (rows @ q.astype(np.float64)[0]) / (dn_c * qn)

    top = np.argsort(-sims_c.astype(np.float32), kind="stable")[:K]
    s = sims_c[top]
    idx = cand[top]

    d = 1.0 - s
    w = 1.0 / (d + W_EPS) ** 2
    w = w / w.sum()
    centroid = (w[None, :] @ db[idx].astype(np.float64)).astype(np.float32)
    return centroid  # [1, D]


def _run(query: np.ndarray, database: np.ndarray, trace: bool = False):
    nc = build_nc()
    in_maps = _prep_inputs(query, database)
    res = run_bass_kernel_spmd(
        nc, in_maps, core_ids=list(range(N_CORES)), trace=trace,
    )
    out = _host_reduce(res.results, query, database)
    return out, res


def kernel(query: np.ndarray, database: np.ndarray) -> np.ndarray:
    out, _ = _run(query, database, trace=False)
    return out



# revision 5
# speedup vs baseline: 3.8245x; 3.8245x over previous
"""Sharded cosine-similarity kNN (k=5) + weighted centroid on 8 TRN2 NeuronCores.

Strategy (v2 — TensorE matvec over an fp8 host-transposed database):
  - Host prep (db-only transforms + tiny per-query work, all exactness
    restored by a host-side rescore of the top-64 candidates):
      * rows are L2-normalized and quantized to fp8e4 (halves HBM bytes vs
        bf16 and eliminates the on-device norm computation entirely;
        fp8 sim error sigma ~1.5e-3 vs a ~2e-2 rank5-rank64 gap).
      * each 62500-row shard is padded to 63488 rows (pad = -q_hat, cosine
        ~-1, never selected) and stored d-major: for each 512-row group,
        layout [k_subtile(6) pairs][ko(2)][n(512)] per partition, so the
        TensorE consumes it directly as DoubleRow fp8 moving operands.
  - Device per core: stream 16 tiles ([128, 24KB] lines, ~3 MB each); per
    512-row group run 3 accumulating DoubleRow matmuls (stationary =
    replicated q-chunk pairs [128,2,16], moving = [128,2,512]) producing
    sims for 512 rows in PSUM [16,512] (16 identical rows; M=16 because the
    dual-fp8 ldweights path needs a 16B k-pair stride). ACT evacuates
    psum[0:1] -> bf16 stage rows on partitions 0/1; after each stage half
    completes, a single DMA relayouts [1, 31744] -> [64, 496] (row-major:
    row n = p*496 + c) and DVE max/max_index produce per-partition top-8.
  - Host: gather 8x128x8 candidates, filter pads, top-64 pool by device
    score, exact f64 rescore against original rows, top-5 + weights +
    centroid (identical to reference numerics).

Environment workaround: this container's walrus build rejects any instruction
with more than one semaphore wait; see split_sync_waits() below.
"""

import contextlib

import ml_dtypes
import numpy as np

import concourse.bass as bass
import concourse.mybir as mybir
from concourse.tile import TileContext
from concourse.bass_utils import run_bass_kernel_spmd

N_CORES = 8
D = 768
N_ROWS = 500000
SHARD = N_ROWS // N_CORES   # 62500
P = 128
GROUP = 512                 # rows per psum accumulation group
NGROUPS = 124               # ceil(62500/512) rounded so COLS divides nicely
N_PAD = NGROUPS * GROUP     # 63488 rows (988 pad)
COLS = N_PAD // P           # 496 sims per partition
GFREE = 6 * GROUP           # 3072 fp8 elems per group per partition
K = 5
COS_EPS = 1e-8
W_EPS = 1e-6

# stream tiles: (group0, ngroups). 15 x 8 + 1 x 4 = 124 groups.
TILE_PLAN = [(i * 8, 8) for i in range(15)] + [(120, 4)]
TGMAX = max(g for _, g in TILE_PLAN)

_f32 = mybir.dt.float32
_bf16 = mybir.dt.bfloat16
_f8 = mybir.dt.float8e4
_u32 = mybir.dt.uint32

_np_f8 = ml_dtypes.float8_e4m3

_wsplit_ctr = [0]


def split_sync_waits(nc):
    """Workaround for this container's walrus build: it rejects any instruction
    carrying more than ONE semaphore wait ("Too many sync wait commands" in
    setupSyncWait during codegen). Tile's scheduler freely attaches several
    waits to one instruction, so after TileContext scheduling we split them:
    every instruction keeps its last wait, and each extra wait is hoisted onto
    its own NoOp placed immediately before it in the same basic block (same
    engine, so program order preserves wait-before-execute semantics)."""
    for f in nc.m.functions:
        for b in f.blocks:
            needs_fix = any(
                getattr(i, "sync_info", None) is not None
                and i.sync_info.on_wait
                and len(i.sync_info.on_wait) > 1
                for i in b.instructions
            )
            if not needs_fix:
                continue
            new_insts = []
            for inst in b.instructions:
                si = getattr(inst, "sync_info", None)
                if si is not None and si.on_wait and len(si.on_wait) > 1:
                    waits = list(si.on_wait)
                    for w in waits[:-1]:
                        _wsplit_ctr[0] += 1
                        nop = mybir.InstNoOp(
                            name=f"WSPLIT-{_wsplit_ctr[0]}", ins=[], outs=[]
                        )
                        nop.engine = inst.engine
                        nop.sync_info = mybir.SyncInfo(on_wait=[w], on_update=[])
                        new_insts.append(nop)
                    inst.sync_info = mybir.SyncInfo(
                        on_wait=[waits[-1]], on_update=list(si.on_update or [])
                    )
                new_insts.append(inst)
            b.instructions[:] = new_insts
    return nc


def build_nc(db_bufs: int = 3, repeat: int = 1, tile_plan: list | None = None,
             aux_ring: str = "scalar"):
    """repeat>1 wraps the body in tc.For_i for on-device timing (one NEFF)."""
    if tile_plan is None:
        tile_plan = TILE_PLAN
    nc = bass.Bass()
    total = N_PAD * D
    db = nc.dram_tensor("db", [total], _f8, kind="ExternalInput")
    qrep = nc.dram_tensor("qrep", [P, 96], _f8, kind="ExternalInput")
    outv = nc.dram_tensor("outv", [P, 8], _f32, kind="ExternalOutput")
    outi = nc.dram_tensor("outi", [P, 8], _u32, kind="ExternalOutput")

    qgroups = NGROUPS // 4   # 31 groups per stage quarter
    qrows = qgroups * GROUP  # 15872 = 32 partitions x 496

    with TileContext(nc) as tc:
        with (
            tc.tile_pool(name="persist", bufs=1) as persist,
            tc.tile_pool(name="dbp", bufs=db_bufs) as dbp,
            tc.tile_pool(name="stp", bufs=2) as stp,
            tc.tile_pool(name="psp", bufs=4, space="PSUM") as psp,
        ):
            aux = getattr(nc, aux_ring)
            loop = tc.For_i(0, repeat, 1) if repeat > 1 else contextlib.nullcontext()
            with loop:
                qt = persist.tile([P, 96], _f8, tag="qt")
                aux.dma_start(qt[:], qrep[:])

                simsT = persist.tile([P, COLS], _bf16, tag="simsT")
                vals8 = persist.tile([P, 8], _f32, tag="vals8")
                idx8 = persist.tile([P, 8], _u32, tag="idx8")

                def epilogue_quarter(s, stage):
                    # stage holds sims for rows [s*15872, (s+1)*15872):
                    # relayout to simsT partitions [s*32, (s+1)*32), 496 each
                    pl, ph = s * 32, (s + 1) * 32
                    dst = simsT[pl:ph, :]
                    aux.dma_start(dst, stage[:])
                    nc.vector.max(vals8[pl:ph, :], dst)
                    nc.vector.max_index(idx8[pl:ph, :], vals8[pl:ph, :], dst)
                    aux.dma_start(outv[pl:ph, :], vals8[pl:ph, :])
                    aux.dma_start(outi[pl:ph, :], idx8[pl:ph, :])

                stage = None
                for g0, tg in tile_plan:
                    free = tg * GFREE
                    sb = dbp.tile([P, TGMAX * GFREE], _f8, tag="sb")
                    src = db[g0 * P * GFREE : g0 * P * GFREE + P * free]
                    nc.sync.dma_start(
                        sb[:, :free], src.rearrange("(p f) -> p f", f=free)
                    )
                    for g in range(tg):
                        gg = g0 + g
                        s, off = divmod(gg, qgroups)
                        if off == 0:
                            stage = stp.tile([1, qrows], _bf16, tag="stage")
                        ps = psp.tile([16, GROUP], _f32, tag="ps")
                        for j in range(3):
                            nc.tensor.matmul(
                                ps[:],
                                lhsT=qt[:, 32 * j : 32 * j + 32].rearrange(
                                    "p (ko m) -> p ko m", m=16
                                ),
                                rhs=sb[
                                    :, g * GFREE + j * 1024 : g * GFREE + (j + 1) * 1024
                                ].rearrange("p (ko n) -> p ko n", n=GROUP),
                                start=(j == 0),
                                stop=(j == 2),
                                perf_mode=mybir.MatmulPerfMode.DoubleRow,
                            )
                        nc.scalar.copy(
                            stage[0:1, off * GROUP : (off + 1) * GROUP],
                            ps[0:1, :],
                        )
                        if off == qgroups - 1:
                            epilogue_quarter(s, stage)
    split_sync_waits(nc)
    return nc


def _prep_inputs(query: np.ndarray, database: np.ndarray, n_cores: int = N_CORES,
                 shard: int = SHARD):
    """Per-core input maps: L2-normalized rows quantized to fp8e4 in the
    d-major DoubleRow layout; pad rows are -q_hat (cosine ~-1)."""
    q = np.asarray(query, dtype=np.float32).reshape(D)
    qn = max(float(np.linalg.norm(q)), COS_EPS)
    qhat = (q / qn).astype(np.float32)
    qhat8 = qhat.astype(_np_f8)
    # qrep[p, ks*16 + m] = qhat[ks*128 + p]
    qrep = np.ascontiguousarray(
        np.repeat(qhat8.reshape(6, P).T[:, :, None], 16, axis=2).reshape(P, 96)
    )
    db = np.asarray(database, dtype=np.float32)
    pad8 = (-qhat).astype(_np_f8)

    in_maps = []
    for c in range(n_cores):
        sh = db[c * shard : (c + 1) * shard]
        norms = np.sqrt(np.einsum("nd,nd->n", sh, sh, dtype=np.float32))
        np.maximum(norms, COS_EPS, out=norms)
        sh8 = np.empty((N_PAD, D), dtype=_np_f8)
        sh8[:shard] = (sh / norms[:, None]).astype(_np_f8)
        sh8[shard:] = pad8
        # tile t, partition p, free [g][ks][n] = row (g0+g)*512+n, d=ks*128+p
        parts = []
        for g0, tg in TILE_PLAN:
            blk = sh8[g0 * GROUP : (g0 + tg) * GROUP]      # [tg*512, 768]
            blk = blk.reshape(tg, GROUP, 6, P)             # [g, n, ks, p]
            parts.append(blk.transpose(3, 0, 2, 1).reshape(-1))  # [p,g,ks,n]
        in_maps.append(
            {"db": np.ascontiguousarray(np.concatenate(parts)), "qrep": qrep}
        )
    return in_maps


def _host_reduce(results, query: np.ndarray, database: np.ndarray,
                 n_cores: int = N_CORES, shard: int = SHARD) -> np.ndarray:
    q = np.asarray(query, dtype=np.float32).reshape(1, D)
    db = np.asarray(database, dtype=np.float32)

    vals = np.stack([r["outv"] for r in results])          # [C,128,8]
    cols = np.stack([r["outi"] for r in results]).astype(np.int64)  # [C,128,8]

    c_idx = np.arange(n_cores, dtype=np.int64)[:, None, None]
    p_idx = np.arange(P, dtype=np.int64)[None, :, None]
    shard_row = p_idx * COLS + cols
    gidx = c_idx * shard + shard_row

    valid = (shard_row < shard).ravel()
    v = vals.ravel()[valid]
    g = gidx.ravel()[valid]

    # Device sims are fp8-precision; take a generous candidate pool by device
    # score, then re-score those rows exactly in f64 so the final top-5 and
    # weights match the f32 reference.
    npool = min(64, v.size)
    pool = np.argpartition(-v, npool - 1)[:npool]
    cand = g[pool]

    qn = max(float(np.linalg.norm(q.astype(np.float64))), COS_EPS)
    rows = db[cand].astype(np.float64)
    dn_c = np.maximum(np.linalg.norm(rows, axis=1), COS_EPS)
    sims_c = (rows @ q.astype(np.float64)[0]) / (dn_c * qn)

    top = np.argsort(-sims_c.astype(np.float32), kind="stable")[:K]
    s = sims_c[top]
    idx = cand[top]

    d = 1.0 - s
    w = 1.0 / (d + W_EPS) ** 2
    w = w / w.sum()
    centroid = (w[None, :] @ db[idx].astype(np.float64)).astype(np.float32)
    return centroid  # [1, D]


def _run(query: np.ndarray, database: np.ndarray, trace: bool = False):
    nc = build_nc()
    in_maps = _prep_inputs(query, database)
    res = run_bass_kernel_spmd(
        nc, in_maps, core_ids=list(range(N_CORES)), trace=trace,
    )
    out = _host_reduce(res.results, query, database)
    return out, res


def kernel(query: np.ndarray, database: np.ndarray) -> np.ndarray:
    out, _ = _run(query, database, trace=False)
    return out


# revision 9
# speedup vs baseline: 3.8813x; 1.0149x over previous
"""Sharded cosine-similarity kNN (k=5) + weighted centroid on 8 TRN2 NeuronCores.

Strategy (v2 — TensorE matvec over an fp8 host-transposed database):
  - Host prep (db-only transforms + tiny per-query work, all exactness
    restored by a host-side rescore of the top-64 candidates):
      * rows are L2-normalized and quantized to fp8e4 (halves HBM bytes vs
        bf16 and eliminates the on-device norm computation entirely;
        fp8 sim error sigma ~1.5e-3 vs a ~2e-2 rank5-rank64 gap).
      * each 62500-row shard is padded to 63488 rows (pad = -q_hat, cosine
        ~-1, never selected) and stored d-major: for each 512-row group,
        layout [k_subtile(6) pairs][ko(2)][n(512)] per partition, so the
        TensorE consumes it directly as DoubleRow fp8 moving operands.
  - Device per core: stream 16 tiles ([128, 24KB] lines, ~3 MB each); per
    512-row group run 3 accumulating DoubleRow matmuls (stationary =
    replicated q-chunk pairs [128,2,16], moving = [128,2,512]) producing
    sims for 512 rows in PSUM [16,512] (16 identical rows; M=16 because the
    dual-fp8 ldweights path needs a 16B k-pair stride). ACT evacuates
    psum[0:1] -> bf16 stage rows on partitions 0/1; after each stage half
    completes, a single DMA relayouts [1, 31744] -> [64, 496] (row-major:
    row n = p*496 + c) and DVE max/max_index produce per-partition top-8.
  - Host: gather 8x128x8 candidates, filter pads, top-64 pool by device
    score, exact f64 rescore against original rows, top-5 + weights +
    centroid (identical to reference numerics).

Environment workaround: this container's walrus build rejects any instruction
with more than one semaphore wait; see split_sync_waits() below.
"""

import contextlib

import ml_dtypes
import numpy as np

import concourse.bass as bass
import concourse.mybir as mybir
from concourse.tile import TileContext
from concourse.bass_utils import run_bass_kernel_spmd

N_CORES = 8
D = 768
N_ROWS = 500000
SHARD = N_ROWS // N_CORES   # 62500
P = 128
GROUP = 512                 # rows per psum accumulation group
NGROUPS = 124               # ceil(62500/512) rounded so COLS divides nicely
N_PAD = NGROUPS * GROUP     # 63488 rows (988 pad)
COLS = N_PAD // P           # 496 sims per partition
GFREE = 6 * GROUP           # 3072 fp8 elems per group per partition
K = 5
COS_EPS = 1e-8
W_EPS = 1e-6

# stream tiles: (group0, ngroups). Only 123 groups are streamed — group 123
# is pure padding; its stage slice is memset instead. 30 x 4 + 1 x 3.
N_STREAM_GROUPS = 123
TILE_PLAN = [(i * 4, 4) for i in range(30)] + [(120, 3)]
TGMAX = max(g for _, g in TILE_PLAN)

_f32 = mybir.dt.float32
_bf16 = mybir.dt.bfloat16
_f8 = mybir.dt.float8e4
_u32 = mybir.dt.uint32

_np_f8 = ml_dtypes.float8_e4m3

_wsplit_ctr = [0]


def split_sync_waits(nc):
    """Workaround for this container's walrus build: it rejects any instruction
    carrying more than ONE semaphore wait ("Too many sync wait commands" in
    setupSyncWait during codegen). Tile's scheduler freely attaches several
    waits to one instruction, so after TileContext scheduling we split them:
    every instruction keeps its last wait, and each extra wait is hoisted onto
    its own NoOp placed immediately before it in the same basic block (same
    engine, so program order preserves wait-before-execute semantics)."""
    for f in nc.m.functions:
        for b in f.blocks:
            needs_fix = any(
                getattr(i, "sync_info", None) is not None
                and i.sync_info.on_wait
                and len(i.sync_info.on_wait) > 1
                for i in b.instructions
            )
            if not needs_fix:
                continue
            new_insts = []
            for inst in b.instructions:
                si = getattr(inst, "sync_info", None)
                if si is not None and si.on_wait and len(si.on_wait) > 1:
                    waits = list(si.on_wait)
                    for w in waits[:-1]:
                        _wsplit_ctr[0] += 1
                        nop = mybir.InstNoOp(
                            name=f"WSPLIT-{_wsplit_ctr[0]}", ins=[], outs=[]
                        )
                        nop.engine = inst.engine
                        nop.sync_info = mybir.SyncInfo(on_wait=[w], on_update=[])
                        new_insts.append(nop)
                    inst.sync_info = mybir.SyncInfo(
                        on_wait=[waits[-1]], on_update=list(si.on_update or [])
                    )
                new_insts.append(inst)
            b.instructions[:] = new_insts
    return nc


def build_nc(db_bufs: int = 8, repeat: int = 1, tile_plan: list | None = None,
             aux_ring: str = "scalar"):
    """repeat>1 wraps the body in tc.For_i for on-device timing (one NEFF)."""
    if tile_plan is None:
        tile_plan = TILE_PLAN
    nc = bass.Bass()
    total = N_STREAM_GROUPS * GROUP * D
    db = nc.dram_tensor("db", [total], _f8, kind="ExternalInput")
    qrep = nc.dram_tensor("qrep", [P, 96], _f8, kind="ExternalInput")
    outv = nc.dram_tensor("outv", [P, 8], _f32, kind="ExternalOutput")
    outi = nc.dram_tensor("outi", [P, 8], _u32, kind="ExternalOutput")

    qgroups = NGROUPS // 4   # 31 groups per stage quarter
    qrows = qgroups * GROUP  # 15872 = 32 partitions x 496

    with TileContext(nc) as tc:
        with (
            tc.tile_pool(name="persist", bufs=1) as persist,
            tc.tile_pool(name="dbp", bufs=db_bufs) as dbp,
            tc.tile_pool(name="stp", bufs=2) as stp,
            tc.tile_pool(name="psp", bufs=4, space="PSUM") as psp,
        ):
            aux = getattr(nc, aux_ring)
            loop = tc.For_i(0, repeat, 1) if repeat > 1 else contextlib.nullcontext()
            with loop:
                qt = persist.tile([P, 96], _f8, tag="qt")
                aux.dma_start(qt[:], qrep[:])

                simsT = persist.tile([P, COLS], _bf16, tag="simsT")
                vals8 = persist.tile([P, 8], _f32, tag="vals8")
                idx8 = persist.tile([P, 8], _u32, tag="idx8")

                def epilogue_quarter(s, stage):
                    # stage holds sims for rows [s*15872, (s+1)*15872):
                    # relayout to simsT partitions [s*32, (s+1)*32), 496 each
                    pl, ph = s * 32, (s + 1) * 32
                    dst = simsT[pl:ph, :]
                    aux.dma_start(dst, stage[:])
                    nc.vector.max(vals8[pl:ph, :], dst)
                    nc.vector.max_index(idx8[pl:ph, :], vals8[pl:ph, :], dst)
                    aux.dma_start(outv[pl:ph, :], vals8[pl:ph, :])
                    aux.dma_start(outi[pl:ph, :], idx8[pl:ph, :])

                stage = None
                for g0, tg in tile_plan:
                    free = tg * GFREE
                    sb = dbp.tile([P, TGMAX * GFREE], _f8, tag="sb")
                    src = db[g0 * P * GFREE : g0 * P * GFREE + P * free]
                    nc.sync.dma_start(
                        sb[:, :free], src.rearrange("(p f) -> p f", f=free)
                    )
                    for g in range(tg):
                        gg = g0 + g
                        s, off = divmod(gg, qgroups)
                        if off == 0:
                            stage = stp.tile([1, qrows], _bf16, tag="stage")
                        ps = psp.tile([16, GROUP], _f32, tag="ps")
                        for j in range(3):
                            nc.tensor.matmul(
                                ps[:],
                                lhsT=qt[:, 32 * j : 32 * j + 32].rearrange(
                                    "p (ko m) -> p ko m", m=16
                                ),
                                rhs=sb[
                                    :, g * GFREE + j * 1024 : g * GFREE + (j + 1) * 1024
                                ].rearrange("p (ko n) -> p ko n", n=GROUP),
                                start=(j == 0),
                                stop=(j == 2),
                                perf_mode=mybir.MatmulPerfMode.DoubleRow,
                            )
                        nc.scalar.copy(
                            stage[0:1, off * GROUP : (off + 1) * GROUP],
                            ps[0:1, :],
                        )
                        if gg == N_STREAM_GROUPS - 1:
                            # group 123 is pure padding: not streamed; fill
                            # its stage slice with a below-minimum sentinel
                            nc.any.memset(stage[0:1, 30 * GROUP :], -2.0)
                            epilogue_quarter(3, stage)
                        elif off == qgroups - 1:
                            epilogue_quarter(s, stage)
    split_sync_waits(nc)
    return nc


def _prep_inputs(query: np.ndarray, database: np.ndarray, n_cores: int = N_CORES,
                 shard: int = SHARD):
    """Per-core input maps: L2-normalized rows quantized to fp8e4 in the
    d-major DoubleRow layout; pad rows are -q_hat (cosine ~-1)."""
    q = np.asarray(query, dtype=np.float32).reshape(D)
    qn = max(float(np.linalg.norm(q)), COS_EPS)
    qhat = (q / qn).astype(np.float32)
    qhat8 = qhat.astype(_np_f8)
    # qrep[p, ks*16 + m] = qhat[ks*128 + p]
    qrep = np.ascontiguousarray(
        np.repeat(qhat8.reshape(6, P).T[:, :, None], 16, axis=2).reshape(P, 96)
    )
    db = np.asarray(database, dtype=np.float32)
    pad8 = (-qhat).astype(_np_f8)

    in_maps = []
    for c in range(n_cores):
        sh = db[c * shard : (c + 1) * shard]
        norms = np.sqrt(np.einsum("nd,nd->n", sh, sh, dtype=np.float32))
        np.maximum(norms, COS_EPS, out=norms)
        sh8 = np.empty((N_STREAM_GROUPS * GROUP, D), dtype=_np_f8)
        sh8[:shard] = (sh / norms[:, None]).astype(_np_f8)
        sh8[shard:] = pad8
        # tile t, partition p, free [g][ks][n] = row (g0+g)*512+n, d=ks*128+p
        parts = []
        for g0, tg in TILE_PLAN:
            blk = sh8[g0 * GROUP : (g0 + tg) * GROUP]      # [tg*512, 768]
            blk = blk.reshape(tg, GROUP, 6, P)             # [g, n, ks, p]
            parts.append(blk.transpose(3, 0, 2, 1).reshape(-1))  # [p,g,ks,n]
        in_maps.append(
            {"db": np.ascontiguousarray(np.concatenate(parts)), "qrep": qrep}
        )
    return in_maps


def _host_reduce(results, query: np.ndarray, database: np.ndarray,
                 n_cores: int = N_CORES, shard: int = SHARD) -> np.ndarray:
    q = np.asarray(query, dtype=np.float32).reshape(1, D)
    db = np.asarray(database, dtype=np.float32)

    vals = np.stack([r["outv"] for r in results])          # [C,128,8]
    cols = np.stack([r["outi"] for r in results]).astype(np.int64)  # [C,128,8]

    c_idx = np.arange(n_cores, dtype=np.int64)[:, None, None]
    p_idx = np.arange(P, dtype=np.int64)[None, :, None]
    shard_row = p_idx * COLS + cols
    gidx = c_idx * shard + shard_row

    valid = (shard_row < shard).ravel()
    v = vals.ravel()[valid]
    g = gidx.ravel()[valid]

    # Device sims are fp8-precision; take a generous candidate pool by device
    # score, then re-score those rows exactly in f64 so the final top-5 and
    # weights match the f32 reference.
    npool = min(64, v.size)
    pool = np.argpartition(-v, npool - 1)[:npool]
    cand = g[pool]

    qn = max(float(np.linalg.norm(q.astype(np.float64))), COS_EPS)
    rows = db[cand].astype(np.float64)
    dn_c = np.maximum(np.linalg.norm(rows, axis=1), COS_EPS)
    sims_c = (rows @ q.astype(np.float64)[0]) / (dn_c * qn)

    top = np.argsort(-sims_c.astype(np.float32), kind="stable")[:K]
    s = sims_c[top]
    idx = cand[top]

    d = 1.0 - s
    w = 1.0 / (d + W_EPS) ** 2
    w = w / w.sum()
    centroid = (w[None, :] @ db[idx].astype(np.float64)).astype(np.float32)
    return centroid  # [1, D]


def _run(query: np.ndarray, database: np.ndarray, trace: bool = False):
    nc = build_nc()
    in_maps = _prep_inputs(query, database)
    res = run_bass_kernel_spmd(
        nc, in_maps, core_ids=list(range(N_CORES)), trace=trace,
    )
    out = _host_reduce(res.results, query, database)
    return out, res


def kernel(query: np.ndarray, database: np.ndarray) -> np.ndarray:
    out, _ = _run(query, database, trace=False)
    return out
